# revision 1
# baseline (speedup 1.0000x reference)
"""DGCN-GRU message passing (nn_DGCNGRU) on 8 Trainium2 NeuronCores.

Strategy (sizes hardcoded for N=50000, K=8, H=128, DEPTH=5, 8 cores):
  - Messages are sharded 6250 rows/core (data-parallel over the message
    dim); the small weights are replicated, all PE matmuls run in fp16.
  - The evolving hidden state h lives in DRAM as a [50000, 128] fp16
    row-major table, rebuilt by an fp16 AllGather of the 8 shards after
    every depth step.
  - The neighbor gather h[bgraph] uses gpsimd dma_gather with a pair trick:
    index = bgraph>>1 (fits the int16 index limit), each descriptor moves
    2 rows (512 B = full DMA descriptor efficiency), transpose=True lands
    the two candidate rows as two [128(h-dim), items] SBUF planes; a 3-op
    scalar_tensor_tensor select against a precomputed parity mask picks
    the right plane per item. Gathers are 896 indices each (SWDGE ring is
    128 descriptors) rotated over 4 SWDGE queues.
  - Compute stays in transposed space [h on partitions, messages on the
    free dim]: r2 = U_r @ h_nei on PE with the depth-invariant r
    precompute added through an accumulated identity matmul; sigmoid/tanh
    on ACT with the torch biases as per-partition bias operands; k-sums
    via a DVE scalar_tensor_tensor tree (sum_h) and 8 accumulated
    identity matmuls (sum of r*h_nei).
  - Step 1 skips the gather entirely (h starts at zero).

kernel(**inputs) takes the full unsharded numpy inputs and returns the
full [50000, 128] float32 output. The Bass program is built and compiled
once per process and reused across calls (it depends only on shapes).
"""


from dataclasses import dataclass

import numpy as np

import concourse.bass as bass
import concourse.mybir as mybir

F16 = mybir.dt.float16
F32 = mybir.dt.float32
F32R = mybir.dt.float32r
I16 = mybir.dt.int16
AF = mybir.ActivationFunctionType
ALU = mybir.AluOpType


@dataclass
class Cfg:
    n_mess: int = 50000
    n_cores: int = 8
    depth: int = 5
    k: int = 8
    h: int = 128
    cn: int = 224          # n's per chunk; 2 gathers of cn*8/2 idxs each

    @property
    def n_loc(self):
        assert self.n_mess % self.n_cores == 0
        return self.n_mess // self.n_cores

    @property
    def n_pad(self):
        return ((self.n_loc + 127) // 128) * 128

    @property
    def items(self):
        return self.n_pad * self.k

    @property
    def chunks(self):
        """List of (n_offset, cn) chunk descriptors covering n_pad."""
        out = []
        off = 0
        while off < self.n_pad:
            cn = min(self.cn, self.n_pad - off)
            assert (cn * self.k) % 128 == 0
            out.append((off, cn))
            off += cn
        return out


def host_inputs(fmess, bgraph, W_z, b_z, W_r, U_r, b_Ur, W_h, b_h, cfg: Cfg):
    """Build the per-core in_map numpy dict."""
    n, h = cfg.n_mess, cfg.h
    nl, npad, k = cfg.n_loc, cfg.n_pad, cfg.k
    fmess = np.asarray(fmess, np.float32)
    bgraph = np.asarray(bgraph)

    shared = {
        "wrT": np.ascontiguousarray(W_r.T).astype(np.float16),
        "wz1T": np.ascontiguousarray(W_z[:, :h].T).astype(np.float16),
        "wh1T": np.ascontiguousarray(W_h[:, :h].T).astype(np.float16),
        "urT": np.ascontiguousarray(U_r.T).astype(np.float16),
        "wz2T": np.ascontiguousarray(W_z[:, h:].T).astype(np.float16),
        "wh2T": np.ascontiguousarray(W_h[:, h:].T).astype(np.float16),
        "ident16": np.eye(h, dtype=np.float16),
        "bz": np.asarray(b_z, np.float32).reshape(h, 1),
        "bur": np.asarray(b_Ur, np.float32).reshape(h, 1),
        "bh": np.asarray(b_h, np.float32).reshape(h, 1),
    }

    in_maps = []
    for c in range(cfg.n_cores):
        sl = slice(c * nl, (c + 1) * nl)
        fT = np.zeros((h, npad), np.float16)
        fT[:, :nl] = fmess[sl].T.astype(np.float16)
        bg = np.zeros((npad, k), np.int64)
        bg[:nl] = bgraph[sl]
        flat = bg.reshape(-1)                       # item stream, n-major
        pidx = (flat >> 1).astype(np.int16)
        idx = np.tile(pidx.reshape(cfg.items // 16, 16).T, (8, 1))
        mask = np.broadcast_to(
            (flat & 1).astype(np.float16), (128, cfg.items)).copy()
        maskcol = np.ones((h, 1), np.float32)
        if c == 0:
            maskcol[:, 0] = 0.0
        in_maps.append({
            "fmessT": fT,
            "idx": idx,
            "mask": mask,
            "maskcol": maskcol,
            **shared,
        })
    return in_maps


def declare_io(nc, cfg: Cfg):
    h, npad = cfg.h, cfg.n_pad
    mk = lambda name, shape, dt: nc.dram_tensor(
        name, list(shape), dt, kind="ExternalInput").ap()
    ins = {
        "fmessT": mk("fmessT", (h, npad), F16),
        "idx": mk("idx", (128, cfg.items // 16), I16),
        "mask": mk("mask", (128, cfg.items), F16),
        "maskcol": mk("maskcol", (h, 1), F32),
        "wrT": mk("wrT", (h, h), F16),
        "wz1T": mk("wz1T", (h, h), F16),
        "wh1T": mk("wh1T", (h, h), F16),
        "urT": mk("urT", (h, h), F16),
        "wz2T": mk("wz2T", (h, h), F16),
        "wh2T": mk("wh2T", (h, h), F16),
        "ident16": mk("ident16", (h, h), F16),
        "bz": mk("bz", (h, 1), F32),
        "bur": mk("bur", (h, 1), F32),
        "bh": mk("bh", (h, 1), F32),
    }
    out = nc.dram_tensor("hT", [h, npad], F32, kind="ExternalOutput").ap()
    return ins, out


def build_gru(tc, out_hT, ins, cfg: Cfg):
    nc = tc.nc
    h, k, npad, nl = cfg.h, cfg.k, cfg.n_pad, cfg.n_loc
    N = cfg.n_mess
    rg = [list(range(cfg.n_cores))]

    # internal DRAM
    table = nc.dram_tensor("table", [N, h], F16, kind="Internal",
                           addr_space="Shared").ap()
    shard = nc.dram_tensor("shard", [nl, h], F16, kind="Internal").ap()
    zpre_d = nc.dram_tensor("zpre_d", [h, npad], F16, kind="Internal").ap()
    hpre_d = nc.dram_tensor("hpre_d", [h, npad], F16, kind="Internal").ap()
    tab_pairs = table.rearrange("(p two) hh -> p (two hh)", two=2)

    with (
        tc.tile_pool(name="stat", bufs=1) as stat,
        tc.tile_pool(name="ld", bufs=2) as ld,
        tc.tile_pool(name="work", bufs=1) as work,
        tc.tile_pool(name="small", bufs=2) as small,
        tc.tile_pool(name="ps_big", bufs=1, space="PSUM") as ps_big,
        tc.tile_pool(name="ps_sm", bufs=1, space="PSUM") as ps_sm,
        tc.tile_pool(name="ps_tr", bufs=2, space="PSUM") as ps_tr,
    ):
        # ---- resident statics ----
        idx_sb = stat.tile([128, cfg.items // 16], I16)
        nc.sync.dma_start(idx_sb[:], ins["idx"][:])
        mask_sb = stat.tile([128, cfg.items], F16)
        nc.sync.dma_start(mask_sb[:], ins["mask"][:])
        rpre_sb = stat.tile([h, npad], F16)
        h16_full = stat.tile([h, npad], F16)
        w = {}
        for name in ("wrT", "wz1T", "wh1T", "wz2T", "wh2T", "urT",
                     "ident16"):
            w[name] = stat.tile([h, h], F16, tag=name, name=name)
            nc.sync.dma_start(w[name][:], ins[name][:])
        for name in ("bz", "bur", "bh", "maskcol"):
            w[name] = stat.tile([h, 1], F32, tag=name, name=name)
            nc.sync.dma_start(w[name][:], ins[name][:])
        urT, i16t = w["urT"], w["ident16"]

        def stt(out, in0, in1, op1):
            nc.vector.scalar_tensor_tensor(out, in0, 0.0, in1, ALU.bypass, op1)

        def shard_write_full():
            """PE-transpose h16_full and DMA rows into the shard."""
            for b in range(0, npad, 128):
                pst = ps_tr.tile([128, 128], F16)
                nc.tensor.transpose(pst[:], h16_full[:, b:b + 128], i16t[:])
                row = small.tile([128, 128], F16, tag="row")
                nc.scalar.activation(row[:], pst[:], AF.Copy)
                rows = max(0, min(nl - b, 128))
                if rows:
                    nc.sync.dma_start(shard[b:b + rows, :], row[:rows, :])

        # ---- phase 0: precomputes + step 1 (h == 0 before the first step) ----
        for (n0, cn) in cfg.chunks:
            csl = slice(n0, n0 + cn)
            fsl = ld.tile([h, cn], F16, tag="fsl")
            nc.sync.dma_start(fsl[:, :cn], ins["fmessT"][:, csl])
            fr = fsl[:, :cn]

            ps = ps_sm.tile([h, cn], F32, tag="psg")
            nc.tensor.matmul(ps[:, :cn], w["wrT"][:], fr,
                             start=True, stop=True)
            nc.scalar.activation(rpre_sb[:, csl], ps[:, :cn], AF.Copy)

            psz = ps_sm.tile([h, cn], F32, tag="psz")
            nc.tensor.matmul(psz[:, :cn], w["wz1T"][:], fr,
                             start=True, stop=True)
            zsl = ld.tile([h, cn], F16, tag="zsl")
            nc.scalar.activation(zsl[:, :cn], psz[:, :cn], AF.Copy)
            nc.sync.dma_start(zpre_d[:, csl], zsl[:, :cn])
            z1 = small.tile([h, cn], F32, tag="z")
            nc.scalar.activation(z1[:, :cn], psz[:, :cn], AF.Sigmoid,
                                 bias=w["bz"][:])

            psh = ps_sm.tile([h, cn], F32, tag="psh")
            nc.tensor.matmul(psh[:, :cn], w["wh1T"][:], fr,
                             start=True, stop=True)
            hsl = ld.tile([h, cn], F16, tag="hsl")
            nc.scalar.activation(hsl[:, :cn], psh[:, :cn], AF.Copy)
            nc.sync.dma_start(hpre_d[:, csl], hsl[:, :cn])
            ph1 = small.tile([h, cn], F32, tag="ph")
            nc.scalar.activation(ph1[:, :cn], psh[:, :cn], AF.Tanh,
                                 bias=w["bh"][:])

            hnew = small.tile([h, cn], F32, tag="hnew")
            stt(hnew[:, :cn], z1[:, :cn], ph1[:, :cn], ALU.mult)
            if n0 == 0:
                stt(hnew[:, 0:1], hnew[:, 0:1], w["maskcol"][:], ALU.mult)
            nc.scalar.activation(h16_full[:, csl], hnew[:, :cn], AF.Copy)

        shard_write_full()
        nc.gpsimd.collective_compute(
            "AllGather", ALU.bypass, replica_groups=rg,
            ins=[shard[:, :]], outs=[table[:, :]])

        # ---- depth steps 2..depth ----
        gq = [0]
        for step in range(1, cfg.depth):
            last = step == cfg.depth - 1
            for ci, (n0, cn) in enumerate(cfg.chunks):
                csl = slice(n0, n0 + cn)
                citems = cn * k
                ioff = n0 * k

                cg = citems // 2
                pair = work.tile([128, 2, 2, cg], F16, tag="pair", bufs=2)
                for g in range(2):
                    nc.gpsimd.dma_gather(
                        out_ap=pair[:, g, :, :],
                        in_ap=tab_pairs,
                        idxs_ap=idx_sb[:, (ioff + g * cg) // 16:
                                       (ioff + (g + 1) * cg) // 16],
                        num_idxs=cg,
                        num_idxs_reg=cg,
                        elem_size=2 * h,
                        transpose=True,
                        queue_num=gq[0] % 4,
                    )
                    gq[0] += 1
                lo = pair[:, :, 0, :]
                hi = pair[:, :, 1, :]
                msl = mask_sb[:, ioff:ioff + citems].rearrange(
                    "p (g c) -> p g c", g=2)

                d = work.tile([128, 2, cg], F16, tag="scr", bufs=2)
                stt(d[:, :, :], hi, lo, ALU.subtract)
                dm = work.tile([128, 2, cg], F16, tag="scr", bufs=2)
                stt(dm[:, :, :], d[:, :, :], msl, ALU.mult)
                hn = work.tile([128, citems], F16, tag="hn", bufs=2)
                stt(hn[:, :citems].rearrange("p (g c) -> p g c", g=2),
                    dm[:, :, :], lo, ALU.add)

                # r2 = U_r @ hn + rpre (broadcast over k), sigmoid
                r16 = work.tile([128, citems], F16, tag="r16", bufs=2)
                for s0 in range(0, citems, 512):
                    sw = min(512, citems - s0)
                    psr = ps_big.tile([128, 512], F32, tag="psr", bufs=2)
                    nc.tensor.matmul(
                        psr[:, :sw], urT[:], hn[:, s0:s0 + sw],
                        start=True, stop=False)
                    nb = sw // k
                    rb = rpre_sb[:, n0 + s0 // k:n0 + s0 // k + nb]
                    rb = rb.rearrange("p (a one) -> p a one", one=1)
                    rb = rb.broadcast_to((128, nb, k))
                    nc.tensor.matmul(psr[:, :sw], i16t[:], rb,
                                     start=False, stop=True)
                    nc.scalar.activation(r16[:, s0:s0 + sw], psr[:, :sw],
                                         AF.Sigmoid, bias=w["bur"][:])

                gated = work.tile([128, citems], F16, tag="gated", bufs=2)
                stt(gated[:, :citems], r16[:, :citems], hn[:, :citems],
                    ALU.mult)

                # sum_h: stt halving tree over k == 8
                v = hn[:, :citems].rearrange("p (a two) -> p a two", two=2)
                t1 = work.tile([128, citems // 2], F16, tag="scr", bufs=2)
                stt(t1[:, :citems // 2], v[:, :, 0], v[:, :, 1], ALU.add)
                v = t1[:, :citems // 2].rearrange("p (a two) -> p a two", two=2)
                t2 = work.tile([128, citems // 4], F16, tag="t2")
                stt(t2[:, :citems // 4], v[:, :, 0], v[:, :, 1], ALU.add)
                v = t2[:, :citems // 4].rearrange("p (a two) -> p a two", two=2)
                sumh = small.tile([h, cn], F32, tag="sumh")
                stt(sumh[:, :cn], v[:, :, 0], v[:, :, 1], ALU.add)
                sumh16 = small.tile([h, cn], F16, tag="sumh16")
                nc.scalar.activation(sumh16[:, :cn], sumh[:, :cn], AF.Copy)

                # sum_gated via 8 accumulated identity matmuls
                gk = gated[:, :citems].rearrange("p (n kk) -> p n kk", kk=k)
                psg = ps_sm.tile([h, cn], F32, tag="psg")
                for kk in range(k):
                    nc.tensor.matmul(psg[:, :cn], i16t[:], gk[:, :, kk],
                                     start=(kk == 0), stop=(kk == k - 1))
                sumg16 = small.tile([h, cn], F16, tag="sumg16")
                nc.scalar.activation(sumg16[:, :cn], psg[:, :cn], AF.Copy)

                # z and pre_h
                zsl = ld.tile([h, cn], F16, tag="zsl")
                nc.sync.dma_start(zsl[:, :cn], zpre_d[:, csl])
                psz = ps_sm.tile([h, cn], F32, tag="psz")
                nc.tensor.matmul(psz[:, :cn], w["wz2T"][:],
                                 sumh16[:, :cn],
                                 start=True, stop=False)
                nc.tensor.matmul(psz[:, :cn], i16t[:],
                                 zsl[:, :cn],
                                 start=False, stop=True)
                z = small.tile([h, cn], F32, tag="z")
                nc.scalar.activation(z[:, :cn], psz[:, :cn], AF.Sigmoid,
                                     bias=w["bz"][:])

                hsl = ld.tile([h, cn], F16, tag="hsl")
                nc.sync.dma_start(hsl[:, :cn], hpre_d[:, csl])
                psh = ps_sm.tile([h, cn], F32, tag="psh")
                nc.tensor.matmul(psh[:, :cn], w["wh2T"][:],
                                 sumg16[:, :cn],
                                 start=True, stop=False)
                nc.tensor.matmul(psh[:, :cn], i16t[:],
                                 hsl[:, :cn],
                                 start=False, stop=True)
                ph = small.tile([h, cn], F32, tag="ph")
                nc.scalar.activation(ph[:, :cn], psh[:, :cn], AF.Tanh,
                                     bias=w["bh"][:])

                # h_new = sum_h + z * (pre_h - sum_h)
                t = small.tile([h, cn], F32, tag="tdiff")
                stt(t[:, :cn], ph[:, :cn], sumh[:, :cn], ALU.subtract)
                tz = small.tile([h, cn], F32, tag="tz")
                stt(tz[:, :cn], t[:, :cn], z[:, :cn], ALU.mult)
                hnew = small.tile([h, cn], F32, tag="hnew")
                stt(hnew[:, :cn], tz[:, :cn], sumh[:, :cn], ALU.add)
                if n0 == 0:
                    stt(hnew[:, 0:1], hnew[:, 0:1], w["maskcol"][:], ALU.mult)

                if last:
                    nc.sync.dma_start(out_hT[:, csl], hnew[:, :cn])
                else:
                    nc.scalar.activation(h16_full[:, csl], hnew[:, :cn], AF.Copy)

            if not last:
                shard_write_full()
                nc.gpsimd.collective_compute(
                    "AllGather", ALU.bypass, replica_groups=rg,
                    ins=[shard[:, :]], outs=[table[:, :]])


CFG = Cfg()


_PROGRAM = None
LAST_RESULTS = None


def _get_program():
    global _PROGRAM
    if _PROGRAM is None:
        import concourse.bacc as bacc
        import concourse.tile as tile
        nc = bacc.Bacc("TRN2", target_bir_lowering=False, debug=False,
                       num_devices=CFG.n_cores, num_swdge_queues=4)
        ins, out = declare_io(nc, CFG)
        with tile.TileContext(nc) as tc:
            build_gru(tc, out, ins, CFG)
        nc.compile()
        _PROGRAM = nc
    return _PROGRAM


def kernel(fmess, bgraph, W_z, b_z, W_r, U_r, b_Ur, W_h, b_h, **_unused):
    global LAST_RESULTS
    import concourse.bass_utils as bass_utils
    cfg = CFG
    fmess_np = np.asarray(fmess)
    out_dtype = fmess_np.dtype
    in_maps = host_inputs(fmess_np, bgraph, W_z, b_z, W_r, U_r, b_Ur,
                          W_h, b_h, cfg)
    nc = _get_program()
    res = bass_utils.run_bass_kernel_spmd(
        nc, in_maps, core_ids=list(range(cfg.n_cores)))
    LAST_RESULTS = res
    parts = []
    for c in range(cfg.n_cores):
        hT = res.results[c]["hT"]
        parts.append(np.ascontiguousarray(hT[:, :cfg.n_loc].T))
    return np.concatenate(parts, axis=0).astype(out_dtype)



# revision 13
# speedup vs baseline: 1.1405x; 1.1405x over previous
"""DGCN-GRU message passing (nn_DGCNGRU) on 8 Trainium2 NeuronCores.

Strategy (sizes hardcoded for N=50000, K=8, H=128, DEPTH=5, 8 cores):
  - Messages sharded 6250 rows/core; small weights replicated; fp16 PE.
  - Evolving h lives in DRAM as a [50000, 128] fp16 row table, rebuilt by
    an fp16 AllGather of the 8 shards after every depth step.
  - Neighbor gather h[bgraph] via gpsimd dma_gather pair trick: idx =
    bgraph>>1 (int16 limit), each descriptor moves the 512B row pair,
    transpose=True lands the two candidate rows as two [128, idx] planes.
  - Pair select: ACT copies the lo plane, DVE copy_predicated overwrites
    with the hi plane under a uint8 parity mask (2 passes on 2 engines
    instead of 3 DVE stt passes).
  - All depth-invariant precomputes (W_r/W_z1/W_h1 @ fmess) are SBUF-
    resident fp16 for the whole kernel; no DRAM round trips per step.
  - Compute stays transposed [h on partitions, messages on free dim]:
    r2 = U_r @ hn in 4 PSUM subtiles then rpre broadcast-accumulated via
    identity matmuls (stationary reloads grouped: urT x4, ident x4,
    wh2 x8, wz2 x1, ident x2 per chunk); sigmoid/tanh on ACT with biases
    as per-partition operands; sum_h via a DVE stt halving tree (fp16);
    sum_gated folded into W_h2 (8 accumulated W_h2 matmuls).
  - Step 1 skips the gather (h == 0).

kernel(**inputs) takes full unsharded numpy inputs, returns the full
[50000, 128] float32 output. The Bass program is compiled once per
process and reused (it depends only on shapes).
"""


from dataclasses import dataclass

import numpy as np

import concourse.bass as bass
import concourse.mybir as mybir

F16 = mybir.dt.float16
F32 = mybir.dt.float32
U8 = mybir.dt.uint8
I16 = mybir.dt.int16
AF = mybir.ActivationFunctionType
ALU = mybir.AluOpType


@dataclass
class Cfg:
    n_mess: int = 50000
    n_cores: int = 8
    depth: int = 5
    k: int = 8
    h: int = 128
    cn: int = 224          # messages per chunk; 2 gathers of cn*k/2 idxs

    @property
    def n_loc(self):
        assert self.n_mess % self.n_cores == 0
        return self.n_mess // self.n_cores

    @property
    def n_pad(self):
        return ((self.n_loc + 127) // 128) * 128

    @property
    def items(self):
        return self.n_pad * self.k

    @property
    def chunks(self):
        out = []
        off = 0
        while off < self.n_pad:
            cn = min(self.cn, self.n_pad - off)
            assert (cn * self.k) % 128 == 0
            out.append((off, cn))
            off += cn
        return out


def host_inputs(fmess, bgraph, W_z, b_z, W_r, U_r, b_Ur, W_h, b_h, cfg: Cfg):
    n, h = cfg.n_mess, cfg.h
    nl, npad, k = cfg.n_loc, cfg.n_pad, cfg.k
    fmess = np.asarray(fmess, np.float32)
    bgraph = np.asarray(bgraph)

    shared = {
        "wrT": np.ascontiguousarray(W_r.T).astype(np.float16),
        "wz1T": np.ascontiguousarray(W_z[:, :h].T).astype(np.float16),
        "wh1T": np.ascontiguousarray(W_h[:, :h].T).astype(np.float16),
        "urT": np.ascontiguousarray(U_r.T).astype(np.float16),
        "wz2T": np.ascontiguousarray(W_z[:, h:].T).astype(np.float16),
        "wh2T": np.ascontiguousarray(W_h[:, h:].T).astype(np.float16),
        "ident16": np.eye(h, dtype=np.float16),
        "bz": np.asarray(b_z, np.float32).reshape(h, 1),
        "bur": np.asarray(b_Ur, np.float32).reshape(h, 1),
        "bh": np.asarray(b_h, np.float32).reshape(h, 1),
    }

    in_maps = []
    for c in range(cfg.n_cores):
        sl = slice(c * nl, (c + 1) * nl)
        fT = np.zeros((h, npad), np.float16)
        fT[:, :nl] = fmess[sl].T.astype(np.float16)
        bg = np.zeros((npad, k), np.int64)
        bg[:nl] = bgraph[sl]
        flat = bg.reshape(-1)                       # item stream, n-major
        pidx = (flat >> 1).astype(np.int16)
        idx = np.tile(pidx.reshape(cfg.items // 16, 16).T, (8, 1))
        mask = np.broadcast_to(
            (flat & 1).astype(np.float16), (128, cfg.items)).copy()
        maskcol = np.ones((h, 1), np.float32)
        if c == 0:
            maskcol[:, 0] = 0.0
        in_maps.append({
            "fmessT": fT,
            "idx": idx,
            "mask": mask,
            "maskcol": maskcol,
            **shared,
        })
    return in_maps


def declare_io(nc, cfg: Cfg):
    h, npad = cfg.h, cfg.n_pad
    mk = lambda name, shape, dt: nc.dram_tensor(
        name, list(shape), dt, kind="ExternalInput").ap()
    ins = {
        "fmessT": mk("fmessT", (h, npad), F16),
        "idx": mk("idx", (128, cfg.items // 16), I16),
        "mask": mk("mask", (128, cfg.items), F16),
        "maskcol": mk("maskcol", (h, 1), F32),
        "wrT": mk("wrT", (h, h), F16),
        "wz1T": mk("wz1T", (h, h), F16),
        "wh1T": mk("wh1T", (h, h), F16),
        "urT": mk("urT", (h, h), F16),
        "wz2T": mk("wz2T", (h, h), F16),
        "wh2T": mk("wh2T", (h, h), F16),
        "ident16": mk("ident16", (h, h), F16),
        "bz": mk("bz", (h, 1), F32),
        "bur": mk("bur", (h, 1), F32),
        "bh": mk("bh", (h, 1), F32),
    }
    out = nc.dram_tensor("hT", [h, npad], F32, kind="ExternalOutput").ap()
    return ins, out


def build_gru(tc, out_hT, ins, cfg: Cfg):
    nc = tc.nc
    h, k, npad, nl = cfg.h, cfg.k, cfg.n_pad, cfg.n_loc
    N = cfg.n_mess
    rg = [list(range(cfg.n_cores))]

    table = nc.dram_tensor("table", [N, h], F16, kind="Internal",
                           addr_space="Shared").ap()
    shard = nc.dram_tensor("shard", [nl, h], F16, kind="Internal").ap()
    tab_pairs = table.rearrange("(p two) hh -> p (two hh)", two=2)

    with (
        tc.tile_pool(name="stat", bufs=1) as stat,
        tc.tile_pool(name="work", bufs=1) as work,
        tc.tile_pool(name="small", bufs=2) as small,
        tc.tile_pool(name="ps_r", bufs=2, space="PSUM") as ps_r,
        tc.tile_pool(name="ps_sm", bufs=1, space="PSUM") as ps_sm,
        tc.tile_pool(name="ps_tr", bufs=1, space="PSUM") as ps_tr,
    ):
        # ---- resident statics ----
        idx_sb = stat.tile([128, cfg.items // 16], I16)
        nc.sync.dma_start(idx_sb[:], ins["idx"][:])
        mask_sb = stat.tile([128, cfg.items], F16)
        nc.sync.dma_start(mask_sb[:], ins["mask"][:])
        rpre_sb = stat.tile([h, npad], F16)
        zpre_sb = stat.tile([h, npad], F16)
        hpre_sb = stat.tile([h, npad], F16)
        h16_full = stat.tile([h, npad], F16)
        w = {}
        for name in ("wrT", "wz1T", "wh1T", "wz2T", "wh2T", "urT",
                     "ident16"):
            w[name] = stat.tile([h, h], F16, tag=name, name=name)
            nc.sync.dma_start(w[name][:], ins[name][:])
        for name in ("bz", "bur", "bh", "maskcol"):
            w[name] = stat.tile([h, 1], F32, tag=name, name=name)
            nc.sync.dma_start(w[name][:], ins[name][:])
        urT, i16t = w["urT"], w["ident16"]

        def stt(out, in0, in1, op1):
            nc.vector.scalar_tensor_tensor(out, in0, 0.0, in1, ALU.bypass, op1)

        def shard_write_full():
            for b in range(0, npad, 128):
                pst = ps_tr.tile([128, 128], F16)
                nc.tensor.transpose(pst[:], h16_full[:, b:b + 128], i16t[:])
                row = small.tile([128, 128], F16, tag="row")
                nc.scalar.activation(row[:], pst[:], AF.Copy)
                rows = max(0, min(nl - b, 128))
                if rows:
                    nc.sync.dma_start(shard[b:b + rows, :], row[:rows, :])

        # ---- phase 0: precomputes + step 1 (h == 0) ----
        for (n0, cn) in cfg.chunks:
            csl = slice(n0, n0 + cn)
            fsl = small.tile([h, cfg.cn], F16, tag="fsl")
            nc.sync.dma_start(fsl[:, :cn], ins["fmessT"][:, csl])
            fr = fsl[:, :cn]

            ps = ps_sm.tile([h, cfg.cn], F32, tag="psz")
            nc.tensor.matmul(ps[:, :cn], w["wrT"][:], fr,
                             start=True, stop=True)
            nc.scalar.activation(rpre_sb[:, csl], ps[:, :cn], AF.Copy)

            psz = ps_sm.tile([h, cfg.cn], F32, tag="psz")
            nc.tensor.matmul(psz[:, :cn], w["wz1T"][:], fr,
                             start=True, stop=True)
            nc.scalar.activation(zpre_sb[:, csl], psz[:, :cn], AF.Copy)
            z1 = small.tile([h, cfg.cn], F32, tag="z")
            nc.scalar.activation(z1[:, :cn], psz[:, :cn], AF.Sigmoid,
                                 bias=w["bz"][:])

            psh = ps_sm.tile([h, cfg.cn], F32, tag="psh")
            nc.tensor.matmul(psh[:, :cn], w["wh1T"][:], fr,
                             start=True, stop=True)
            nc.scalar.activation(hpre_sb[:, csl], psh[:, :cn], AF.Copy)
            ph1 = small.tile([h, cfg.cn], F32, tag="ph")
            nc.scalar.activation(ph1[:, :cn], psh[:, :cn], AF.Tanh,
                                 bias=w["bh"][:])

            hnew = small.tile([h, cfg.cn], F32, tag="hnew")
            stt(hnew[:, :cn], z1[:, :cn], ph1[:, :cn], ALU.mult)
            if n0 == 0:
                stt(hnew[:, 0:1], hnew[:, 0:1], w["maskcol"][:], ALU.mult)
            nc.scalar.activation(h16_full[:, csl], hnew[:, :cn], AF.Copy)

        shard_write_full()
        nc.gpsimd.collective_compute(
            "AllGather", ALU.bypass, replica_groups=rg,
            ins=[shard[:, :]], outs=[table[:, :]])

        # ---- depth steps 2..depth ----
        gq = [0]
        for step in range(1, cfg.depth):
            last = step == cfg.depth - 1
            for ci, (n0, cn) in enumerate(cfg.chunks):
                csl = slice(n0, n0 + cn)
                citems = cn * k
                ioff = n0 * k
                cg = citems // 2

                pair = work.tile([128, 2, 2, cg], F16, tag="pair", bufs=2)
                for g in range(2):
                    nc.gpsimd.dma_gather(
                        out_ap=pair[:, g, :, :],
                        in_ap=tab_pairs,
                        idxs_ap=idx_sb[:, (ioff + g * cg) // 16:
                                       (ioff + (g + 1) * cg) // 16],
                        num_idxs=cg,
                        num_idxs_reg=cg,
                        elem_size=2 * h,
                        transpose=True,
                        queue_num=gq[0] % 4,
                    )
                    gq[0] += 1
                lo = pair[:, :, 0, :]
                hi = pair[:, :, 1, :]
                msl = mask_sb[:, ioff:ioff + citems].rearrange(
                    "p (g c) -> p g c", g=2)

                # select: ACT copies lo, DVE overwrites hi where parity=1
                hn = work.tile([128, citems], F16, tag="hn", bufs=2)
                hn_g = hn[:, :citems].rearrange("p (g c) -> p g c", g=2)
                d = work.tile([128, 2, cg], F16, tag="scr", bufs=1)
                stt(d[:, :, :], hi, lo, ALU.subtract)
                stt(d[:, :, :], d[:, :, :], msl, ALU.mult)
                stt(hn_g, d[:, :, :], lo, ALU.add)

                # r2 = U_r @ hn + rpre (broadcast over k), sigmoid
                r16 = work.tile([128, citems], F16, tag="r16", bufs=2)
                for s0 in range(0, citems, 448):
                    sw = min(448, citems - s0)
                    psr = ps_r.tile([128, 2, 448], F32, tag="psr")
                    psr = psr.rearrange("p a b -> p (a b)")
                    nc.tensor.matmul(
                        psr[:, :sw], urT[:], hn[:, s0:s0 + sw],
                        start=True, stop=False)
                    nb = sw // k
                    rb = rpre_sb[:, n0 + s0 // k:n0 + s0 // k + nb]
                    rb = rb.rearrange("p (a one) -> p a one", one=1)
                    rb = rb.broadcast_to((128, nb, k))
                    nc.tensor.matmul(psr[:, :sw], i16t[:], rb,
                                     start=False, stop=True)
                    nc.scalar.activation(r16[:, s0:s0 + sw], psr[:, :sw],
                                         AF.Sigmoid, bias=w["bur"][:])

                gated = work.tile([128, citems], F16, tag="gated", bufs=1)
                stt(gated[:, :citems], r16[:, :citems], hn[:, :citems],
                    ALU.mult)

                # sum_gated via 8 accumulated identity matmuls
                gk = gated[:, :citems].rearrange("p (n kk) -> p n kk", kk=k)
                psg = ps_sm.tile([h, cfg.cn], F32, tag="psg")
                for kk in range(k):
                    nc.tensor.matmul(psg[:, :cn], i16t[:], gk[:, :, kk],
                                     start=(kk == 0), stop=(kk == k - 1))
                sumg16 = small.tile([h, cfg.cn], F16, tag="sumg16")
                nc.scalar.activation(sumg16[:, :cn], psg[:, :cn], AF.Copy)
                psh = ps_sm.tile([h, cfg.cn], F32, tag="psh")
                nc.tensor.matmul(psh[:, :cn], w["wh2T"][:], sumg16[:, :cn],
                                 start=True, stop=False)
                nc.tensor.matmul(psh[:, :cn], i16t[:], hpre_sb[:, csl],
                                 start=False, stop=True)
                ph = small.tile([h, cfg.cn], F32, tag="ph")
                nc.scalar.activation(ph[:, :cn], psh[:, :cn], AF.Tanh,
                                     bias=w["bh"][:])

                # sum_h: stt halving tree over k == 8 (fp16 out)
                v = hn[:, :citems].rearrange("p (a two) -> p a two", two=2)
                t1 = work.tile([128, citems // 2], F16, tag="t1", bufs=1)
                stt(t1[:, :citems // 2], v[:, :, 0], v[:, :, 1], ALU.add)
                v = t1[:, :citems // 2].rearrange("p (a two) -> p a two",
                                                  two=2)
                t2 = work.tile([128, citems // 4], F16, tag="t2", bufs=1)
                stt(t2[:, :citems // 4], v[:, :, 0], v[:, :, 1], ALU.add)
                v = t2[:, :citems // 4].rearrange("p (a two) -> p a two",
                                                  two=2)
                sumh16 = small.tile([h, cfg.cn], F16, tag="sumh16")
                stt(sumh16[:, :cn], v[:, :, 0], v[:, :, 1], ALU.add)

                # z path
                psz = ps_sm.tile([h, cfg.cn], F32, tag="psz")
                nc.tensor.matmul(psz[:, :cn], w["wz2T"][:], sumh16[:, :cn],
                                 start=True, stop=False)
                nc.tensor.matmul(psz[:, :cn], i16t[:], zpre_sb[:, csl],
                                 start=False, stop=True)
                z = small.tile([h, cfg.cn], F32, tag="z")
                nc.scalar.activation(z[:, :cn], psz[:, :cn], AF.Sigmoid,
                                     bias=w["bz"][:])

                # h_new = sum_h + z * (pre_h - sum_h)
                t = small.tile([h, cfg.cn], F32, tag="tdiff")
                stt(t[:, :cn], ph[:, :cn], sumh16[:, :cn], ALU.subtract)
                tz = small.tile([h, cfg.cn], F32, tag="tz")
                stt(tz[:, :cn], t[:, :cn], z[:, :cn], ALU.mult)
                hnew = small.tile([h, cfg.cn], F32, tag="hnew")
                stt(hnew[:, :cn], tz[:, :cn], sumh16[:, :cn], ALU.add)
                if n0 == 0:
                    stt(hnew[:, 0:1], hnew[:, 0:1], w["maskcol"][:], ALU.mult)

                if last:
                    nc.sync.dma_start(out_hT[:, csl], hnew[:, :cn])
                else:
                    nc.scalar.activation(h16_full[:, csl], hnew[:, :cn],
                                         AF.Copy)

            if not last:
                shard_write_full()
                nc.gpsimd.collective_compute(
                    "AllGather", ALU.bypass, replica_groups=rg,
                    ins=[shard[:, :]], outs=[table[:, :]])


CFG = Cfg()


_PROGRAM = None
LAST_RESULTS = None


def _get_program():
    global _PROGRAM
    if _PROGRAM is None:
        import concourse.bacc as bacc
        import concourse.tile as tile
        nc = bacc.Bacc("TRN2", target_bir_lowering=False, debug=False,
                       num_devices=CFG.n_cores, num_swdge_queues=4)
        ins, out = declare_io(nc, CFG)
        with tile.TileContext(nc) as tc:
            build_gru(tc, out, ins, CFG)
        nc.compile()
        _PROGRAM = nc
    return _PROGRAM


def kernel(fmess, bgraph, W_z, b_z, W_r, U_r, b_Ur, W_h, b_h, **_unused):
    global LAST_RESULTS
    import concourse.bass_utils as bass_utils
    cfg = CFG
    fmess_np = np.asarray(fmess)
    out_dtype = fmess_np.dtype
    in_maps = host_inputs(fmess_np, bgraph, W_z, b_z, W_r, U_r, b_Ur,
                          W_h, b_h, cfg)
    nc = _get_program()
    res = bass_utils.run_bass_kernel_spmd(
        nc, in_maps, core_ids=list(range(cfg.n_cores)))
    LAST_RESULTS = res
    parts = []
    for c in range(cfg.n_cores):
        hT = res.results[c]["hT"]
        parts.append(np.ascontiguousarray(hT[:, :cfg.n_loc].T))
    return np.concatenate(parts, axis=0).astype(out_dtype)


# revision 14
# speedup vs baseline: 1.2430x; 1.0899x over previous
"""DGCN-GRU message passing (nn_DGCNGRU) on 8 Trainium2 NeuronCores.

Strategy (sizes hardcoded for N=50000, K=8, H=128, DEPTH=5, 8 cores):
  - Messages sharded 6250 rows/core; small weights replicated; fp16 PE.
  - Evolving h lives in DRAM as a [50000, 128] fp16 row table, rebuilt by
    an fp16 AllGather of the 8 shards after every depth step.
  - Neighbor gather h[bgraph] via gpsimd dma_gather pair trick: idx =
    bgraph>>1 (int16 limit), each descriptor moves the 512B row pair,
    transpose=True lands the two candidate rows as two [128, idx] planes.
  - Pair select: ACT copies the lo plane, DVE copy_predicated overwrites
    with the hi plane under a uint8 parity mask (2 passes on 2 engines
    instead of 3 DVE stt passes).
  - All depth-invariant precomputes (W_r/W_z1/W_h1 @ fmess) are SBUF-
    resident fp16 for the whole kernel; no DRAM round trips per step.
  - Compute stays transposed [h on partitions, messages on free dim]:
    r2 = U_r @ hn in 4 PSUM subtiles then rpre broadcast-accumulated via
    identity matmuls (stationary reloads grouped: urT x4, ident x4,
    wh2 x8, wz2 x1, ident x2 per chunk); sigmoid/tanh on ACT with biases
    as per-partition operands; sum_h via a DVE stt halving tree (fp16);
    sum_gated folded into W_h2 (8 accumulated W_h2 matmuls).
  - Step 1 skips the gather (h == 0).

kernel(**inputs) takes full unsharded numpy inputs, returns the full
[50000, 128] float32 output. The Bass program is compiled once per
process and reused (it depends only on shapes).
"""


from dataclasses import dataclass

import numpy as np

import concourse.bass as bass
import concourse.mybir as mybir

F16 = mybir.dt.float16
F32 = mybir.dt.float32
U8 = mybir.dt.uint8
I16 = mybir.dt.int16
AF = mybir.ActivationFunctionType
ALU = mybir.AluOpType


@dataclass
class Cfg:
    n_mess: int = 50000
    n_cores: int = 8
    depth: int = 5
    k: int = 8
    h: int = 128
    cn: int = 224          # messages per chunk; 2 gathers of cn*k/2 idxs

    @property
    def n_loc(self):
        assert self.n_mess % self.n_cores == 0
        return self.n_mess // self.n_cores

    @property
    def n_pad(self):
        return ((self.n_loc + 127) // 128) * 128

    @property
    def items(self):
        return self.n_pad * self.k

    @property
    def chunks(self):
        out = []
        off = 0
        while off < self.n_pad:
            cn = min(self.cn, self.n_pad - off)
            assert (cn * self.k) % 128 == 0
            out.append((off, cn))
            off += cn
        return out


def host_inputs(fmess, bgraph, W_z, b_z, W_r, U_r, b_Ur, W_h, b_h, cfg: Cfg):
    n, h = cfg.n_mess, cfg.h
    nl, npad, k = cfg.n_loc, cfg.n_pad, cfg.k
    fmess = np.asarray(fmess, np.float32)
    bgraph = np.asarray(bgraph)

    shared = {
        "wrT": np.ascontiguousarray(W_r.T).astype(np.float16),
        "wz1T": np.ascontiguousarray(W_z[:, :h].T).astype(np.float16),
        "wh1T": np.ascontiguousarray(W_h[:, :h].T).astype(np.float16),
        "urT": np.ascontiguousarray(U_r.T).astype(np.float16),
        "wz2T": np.ascontiguousarray(W_z[:, h:].T).astype(np.float16),
        "wh2T": np.ascontiguousarray(W_h[:, h:].T).astype(np.float16),
        "ident16": np.eye(h, dtype=np.float16),
        "bz": np.asarray(b_z, np.float32).reshape(h, 1),
        "bur": np.asarray(b_Ur, np.float32).reshape(h, 1),
        "bh": np.asarray(b_h, np.float32).reshape(h, 1),
    }

    in_maps = []
    for c in range(cfg.n_cores):
        sl = slice(c * nl, (c + 1) * nl)
        fT = np.zeros((h, npad), np.float16)
        fT[:, :nl] = fmess[sl].T.astype(np.float16)
        bg = np.zeros((npad, k), np.int64)
        bg[:nl] = bgraph[sl]
        flat = bg.reshape(-1)                       # item stream, n-major
        pidx = (flat >> 1).astype(np.int16)
        idx = np.tile(pidx.reshape(cfg.items // 16, 16).T, (8, 1))
        mask = np.broadcast_to(
            (flat & 1).astype(np.uint8), (128, cfg.items)).copy()
        maskcol = np.ones((h, 1), np.float32)
        if c == 0:
            maskcol[:, 0] = 0.0
        in_maps.append({
            "fmessT": fT,
            "idx": idx,
            "mask": mask,
            "maskcol": maskcol,
            **shared,
        })
    return in_maps


def declare_io(nc, cfg: Cfg):
    h, npad = cfg.h, cfg.n_pad
    mk = lambda name, shape, dt: nc.dram_tensor(
        name, list(shape), dt, kind="ExternalInput").ap()
    ins = {
        "fmessT": mk("fmessT", (h, npad), F16),
        "idx": mk("idx", (128, cfg.items // 16), I16),
        "mask": mk("mask", (128, cfg.items), U8),
        "maskcol": mk("maskcol", (h, 1), F32),
        "wrT": mk("wrT", (h, h), F16),
        "wz1T": mk("wz1T", (h, h), F16),
        "wh1T": mk("wh1T", (h, h), F16),
        "urT": mk("urT", (h, h), F16),
        "wz2T": mk("wz2T", (h, h), F16),
        "wh2T": mk("wh2T", (h, h), F16),
        "ident16": mk("ident16", (h, h), F16),
        "bz": mk("bz", (h, 1), F32),
        "bur": mk("bur", (h, 1), F32),
        "bh": mk("bh", (h, 1), F32),
    }
    out = nc.dram_tensor("hT", [h, npad], F32, kind="ExternalOutput").ap()
    return ins, out


def build_gru(tc, out_hT, ins, cfg: Cfg):
    nc = tc.nc
    h, k, npad, nl = cfg.h, cfg.k, cfg.n_pad, cfg.n_loc
    N = cfg.n_mess
    rg = [list(range(cfg.n_cores))]

    table = nc.dram_tensor("table", [N, h], F16, kind="Internal",
                           addr_space="Shared").ap()
    shard = nc.dram_tensor("shard", [nl, h], F16, kind="Internal").ap()
    tab_pairs = table.rearrange("(p two) hh -> p (two hh)", two=2)

    with (
        tc.tile_pool(name="stat", bufs=1) as stat,
        tc.tile_pool(name="work", bufs=1) as work,
        tc.tile_pool(name="small", bufs=2) as small,
        tc.tile_pool(name="ps_r", bufs=2, space="PSUM") as ps_r,
        tc.tile_pool(name="ps_sm", bufs=1, space="PSUM") as ps_sm,
        tc.tile_pool(name="ps_tr", bufs=1, space="PSUM") as ps_tr,
    ):
        # ---- resident statics ----
        idx_sb = stat.tile([128, cfg.items // 16], I16)
        nc.sync.dma_start(idx_sb[:], ins["idx"][:])
        mask_sb = stat.tile([128, cfg.items], U8)
        nc.sync.dma_start(mask_sb[:], ins["mask"][:])
        rpre_sb = stat.tile([h, npad], F16)
        zpre_sb = stat.tile([h, npad], F16)
        hpre_sb = stat.tile([h, npad], F16)
        h16_full = stat.tile([h, npad], F16)
        w = {}
        for name in ("wrT", "wz1T", "wh1T", "wz2T", "wh2T", "urT",
                     "ident16"):
            w[name] = stat.tile([h, h], F16, tag=name, name=name)
            nc.sync.dma_start(w[name][:], ins[name][:])
        for name in ("bz", "bur", "bh", "maskcol"):
            w[name] = stat.tile([h, 1], F32, tag=name, name=name)
            nc.sync.dma_start(w[name][:], ins[name][:])
        urT, i16t = w["urT"], w["ident16"]

        def stt(out, in0, in1, op1):
            nc.vector.scalar_tensor_tensor(out, in0, 0.0, in1, ALU.bypass, op1)

        def shard_write_full():
            for b in range(0, npad, 128):
                pst = ps_tr.tile([128, 128], F16)
                nc.tensor.transpose(pst[:], h16_full[:, b:b + 128], i16t[:])
                row = small.tile([128, 128], F16, tag="row")
                nc.scalar.activation(row[:], pst[:], AF.Copy)
                rows = max(0, min(nl - b, 128))
                if rows:
                    nc.sync.dma_start(shard[b:b + rows, :], row[:rows, :])

        # ---- phase 0: precomputes + step 1 (h == 0) ----
        for (n0, cn) in cfg.chunks:
            csl = slice(n0, n0 + cn)
            fsl = small.tile([h, cfg.cn], F16, tag="fsl")
            nc.sync.dma_start(fsl[:, :cn], ins["fmessT"][:, csl])
            fr = fsl[:, :cn]

            ps = ps_sm.tile([h, cfg.cn], F32, tag="psz")
            nc.tensor.matmul(ps[:, :cn], w["wrT"][:], fr,
                             start=True, stop=True)
            nc.scalar.activation(rpre_sb[:, csl], ps[:, :cn], AF.Copy)

            psz = ps_sm.tile([h, cfg.cn], F32, tag="psz")
            nc.tensor.matmul(psz[:, :cn], w["wz1T"][:], fr,
                             start=True, stop=True)
            nc.scalar.activation(zpre_sb[:, csl], psz[:, :cn], AF.Copy)
            z1 = small.tile([h, cfg.cn], F32, tag="z")
            nc.scalar.activation(z1[:, :cn], psz[:, :cn], AF.Sigmoid,
                                 bias=w["bz"][:])

            psh = ps_sm.tile([h, cfg.cn], F32, tag="psh")
            nc.tensor.matmul(psh[:, :cn], w["wh1T"][:], fr,
                             start=True, stop=True)
            nc.scalar.activation(hpre_sb[:, csl], psh[:, :cn], AF.Copy)
            ph1 = small.tile([h, cfg.cn], F32, tag="ph")
            nc.scalar.activation(ph1[:, :cn], psh[:, :cn], AF.Tanh,
                                 bias=w["bh"][:])

            hnew = small.tile([h, cfg.cn], F32, tag="hnew")
            stt(hnew[:, :cn], z1[:, :cn], ph1[:, :cn], ALU.mult)
            if n0 == 0:
                stt(hnew[:, 0:1], hnew[:, 0:1], w["maskcol"][:], ALU.mult)
            nc.scalar.activation(h16_full[:, csl], hnew[:, :cn], AF.Copy)

        shard_write_full()
        nc.gpsimd.collective_compute(
            "AllGather", ALU.bypass, replica_groups=rg,
            ins=[shard[:, :]], outs=[table[:, :]])

        # ---- depth steps 2..depth ----
        gq = [0]
        for step in range(1, cfg.depth):
            last = step == cfg.depth - 1
            for ci, (n0, cn) in enumerate(cfg.chunks):
                csl = slice(n0, n0 + cn)
                citems = cn * k
                ioff = n0 * k
                cg = citems // 2

                pair = work.tile([128, 2, 2, cg], F16, tag="pair", bufs=2)
                for g in range(2):
                    nc.gpsimd.dma_gather(
                        out_ap=pair[:, g, :, :],
                        in_ap=tab_pairs,
                        idxs_ap=idx_sb[:, (ioff + g * cg) // 16:
                                       (ioff + (g + 1) * cg) // 16],
                        num_idxs=cg,
                        num_idxs_reg=cg,
                        elem_size=2 * h,
                        transpose=True,
                        queue_num=gq[0] % 4,
                    )
                    gq[0] += 1
                lo = pair[:, :, 0, :]
                hi = pair[:, :, 1, :]
                msl = mask_sb[:, ioff:ioff + citems].rearrange(
                    "p (g c) -> p g c", g=2)

                # select: ACT copies lo, DVE overwrites hi where parity=1
                hn = work.tile([128, citems], F16, tag="hn", bufs=2)
                hn_g = hn[:, :citems].rearrange("p (g c) -> p g c", g=2)
                nc.scalar.activation(hn_g, lo, AF.Copy)
                nc.vector.copy_predicated(hn_g, msl, hi)

                # r2 = U_r @ hn (+ rpre broadcast); 4 bank-aligned
                # psum subtiles, stationary loads batched (urT x4 then
                # ident x4)
                sw = citems // 4         # 448
                nbw = sw // k            # 56
                psr_a = ps_r.tile([128, 2, 512], F32, tag="psr")
                psr_b = ps_r.tile([128, 2, 512], F32, tag="psr")
                sub = lambda j: (psr_a if j < 2 else psr_b)[:, j % 2, :sw]
                for j in range(4):
                    nc.tensor.matmul(sub(j), urT[:],
                                     hn[:, j * sw:(j + 1) * sw],
                                     start=True, stop=False)
                for j in range(4):
                    rb = rpre_sb[:, n0 + j * nbw:n0 + (j + 1) * nbw]
                    rb = rb.rearrange("p (a one) -> p a one", one=1)
                    rb = rb.broadcast_to((128, nbw, k))
                    nc.tensor.matmul(sub(j), i16t[:], rb,
                                     start=False, stop=True)
                r16 = work.tile([128, citems], F16, tag="r16", bufs=2)
                for j in range(4):
                    nc.scalar.activation(r16[:, j * sw:(j + 1) * sw],
                                         sub(j),
                                         AF.Sigmoid, bias=w["bur"][:])

                gated = work.tile([128, citems], F16, tag="gated", bufs=2)
                stt(gated[:, :citems], r16[:, :citems], hn[:, :citems],
                    ALU.mult)

                # sum_gated folded into W_h2: 8 accumulated matmuls
                gk = gated[:, :citems].rearrange("p (n kk) -> p n kk", kk=k)
                psh = ps_sm.tile([h, cfg.cn], F32, tag="psh")
                for kk in range(k):
                    nc.tensor.matmul(psh[:, :cn], w["wh2T"][:], gk[:, :, kk],
                                     start=(kk == 0), stop=False)
                nc.tensor.matmul(psh[:, :cn], i16t[:], hpre_sb[:, csl],
                                 start=False, stop=True)
                ph = small.tile([h, cfg.cn], F32, tag="ph")
                nc.scalar.activation(ph[:, :cn], psh[:, :cn], AF.Tanh,
                                     bias=w["bh"][:])

                # sum_h: stt halving tree over k == 8 (fp16 out)
                v = hn[:, :citems].rearrange("p (a two) -> p a two", two=2)
                t1 = work.tile([128, citems // 2], F16, tag="t1", bufs=2)
                stt(t1[:, :citems // 2], v[:, :, 0], v[:, :, 1], ALU.add)
                v = t1[:, :citems // 2].rearrange("p (a two) -> p a two",
                                                  two=2)
                t2 = work.tile([128, citems // 4], F16, tag="t2", bufs=2)
                stt(t2[:, :citems // 4], v[:, :, 0], v[:, :, 1], ALU.add)
                v = t2[:, :citems // 4].rearrange("p (a two) -> p a two",
                                                  two=2)
                sumh16 = small.tile([h, cfg.cn], F16, tag="sumh16")
                stt(sumh16[:, :cn], v[:, :, 0], v[:, :, 1], ALU.add)

                # z path
                psz = ps_sm.tile([h, cfg.cn], F32, tag="psz")
                nc.tensor.matmul(psz[:, :cn], w["wz2T"][:], sumh16[:, :cn],
                                 start=True, stop=False)
                nc.tensor.matmul(psz[:, :cn], i16t[:], zpre_sb[:, csl],
                                 start=False, stop=True)
                z = small.tile([h, cfg.cn], F32, tag="z")
                nc.scalar.activation(z[:, :cn], psz[:, :cn], AF.Sigmoid,
                                     bias=w["bz"][:])

                # h_new = sum_h + z * (pre_h - sum_h)
                t = small.tile([h, cfg.cn], F32, tag="tdiff")
                stt(t[:, :cn], ph[:, :cn], sumh16[:, :cn], ALU.subtract)
                tz = small.tile([h, cfg.cn], F32, tag="tz")
                stt(tz[:, :cn], t[:, :cn], z[:, :cn], ALU.mult)
                hnew = small.tile([h, cfg.cn], F32, tag="hnew")
                stt(hnew[:, :cn], tz[:, :cn], sumh16[:, :cn], ALU.add)
                if n0 == 0:
                    stt(hnew[:, 0:1], hnew[:, 0:1], w["maskcol"][:], ALU.mult)

                if last:
                    nc.sync.dma_start(out_hT[:, csl], hnew[:, :cn])
                else:
                    nc.scalar.activation(h16_full[:, csl], hnew[:, :cn],
                                         AF.Copy)

            if not last:
                shard_write_full()
                nc.gpsimd.collective_compute(
                    "AllGather", ALU.bypass, replica_groups=rg,
                    ins=[shard[:, :]], outs=[table[:, :]])


CFG = Cfg()


_PROGRAM = None
LAST_RESULTS = None


def _get_program():
    global _PROGRAM
    if _PROGRAM is None:
        import concourse.bacc as bacc
        import concourse.tile as tile
        nc = bacc.Bacc("TRN2", target_bir_lowering=False, debug=False,
                       num_devices=CFG.n_cores, num_swdge_queues=4)
        ins, out = declare_io(nc, CFG)
        with tile.TileContext(nc) as tc:
            build_gru(tc, out, ins, CFG)
        nc.compile()
        _PROGRAM = nc
    return _PROGRAM


def kernel(fmess, bgraph, W_z, b_z, W_r, U_r, b_Ur, W_h, b_h, **_unused):
    global LAST_RESULTS
    import concourse.bass_utils as bass_utils
    cfg = CFG
    fmess_np = np.asarray(fmess)
    out_dtype = fmess_np.dtype
    in_maps = host_inputs(fmess_np, bgraph, W_z, b_z, W_r, U_r, b_Ur,
                          W_h, b_h, cfg)
    nc = _get_program()
    res = bass_utils.run_bass_kernel_spmd(
        nc, in_maps, core_ids=list(range(cfg.n_cores)))
    LAST_RESULTS = res
    parts = []
    for c in range(cfg.n_cores):
        hT = res.results[c]["hT"]
        parts.append(np.ascontiguousarray(hT[:, :cfg.n_loc].T))
    return np.concatenate(parts, axis=0).astype(out_dtype)


# revision 16
# speedup vs baseline: 1.4248x; 1.1462x over previous
"""DGCN-GRU message passing (nn_DGCNGRU) on 8 Trainium2 NeuronCores.

Strategy (sizes hardcoded for N=50000, K=8, H=128, DEPTH=5, 8 cores):
  - Messages sharded 6250 rows/core; small weights replicated; fp16 PE.
  - Evolving h lives in DRAM as a [50000, 128] fp16 row table, rebuilt by
    an fp16 AllGather of the 8 shards after every depth step.
  - Neighbor gather h[bgraph] via gpsimd dma_gather pair trick: idx =
    bgraph>>1 (int16 limit), each descriptor moves the 512B row pair,
    transpose=True lands the two candidate rows as two [128, idx] planes.
  - Pair select: ACT copies the lo plane, DVE copy_predicated overwrites
    with the hi plane under a uint8 parity mask (2 passes on 2 engines
    instead of 3 DVE stt passes).
  - All depth-invariant precomputes (W_r/W_z1/W_h1 @ fmess) are SBUF-
    resident fp16 for the whole kernel; no DRAM round trips per step.
  - Compute stays transposed [h on partitions, messages on free dim]:
    r2 = U_r @ hn in 4 PSUM subtiles then rpre broadcast-accumulated via
    identity matmuls (stationary reloads grouped: urT x4, ident x4,
    wh2 x8, wz2 x1, ident x2 per chunk); sigmoid/tanh on ACT with biases
    as per-partition operands; sum_h via a DVE stt halving tree (fp16);
    sum_gated folded into W_h2 (8 accumulated W_h2 matmuls).
  - Step 1 skips the gather (h == 0).

kernel(**inputs) takes full unsharded numpy inputs, returns the full
[50000, 128] float32 output. The Bass program is compiled once per
process and reused (it depends only on shapes).
"""


from dataclasses import dataclass

import numpy as np

import concourse.bass as bass
import concourse.mybir as mybir

F16 = mybir.dt.float16
F32 = mybir.dt.float32
U8 = mybir.dt.uint8
I16 = mybir.dt.int16
AF = mybir.ActivationFunctionType
ALU = mybir.AluOpType


@dataclass
class Cfg:
    n_mess: int = 50000
    n_cores: int = 8
    depth: int = 5
    k: int = 8
    h: int = 128
    cn: int = 224          # messages per chunk; 2 gathers of cn*k/2 idxs

    @property
    def n_loc(self):
        assert self.n_mess % self.n_cores == 0
        return self.n_mess // self.n_cores

    @property
    def n_pad(self):
        return ((self.n_loc + 127) // 128) * 128

    @property
    def items(self):
        return self.n_pad * self.k

    @property
    def chunks(self):
        out = []
        off = 0
        while off < self.n_pad:
            cn = min(self.cn, self.n_pad - off)
            assert (cn * self.k) % 128 == 0
            out.append((off, cn))
            off += cn
        return out


def host_inputs(fmess, bgraph, W_z, b_z, W_r, U_r, b_Ur, W_h, b_h, cfg: Cfg):
    n, h = cfg.n_mess, cfg.h
    nl, npad, k = cfg.n_loc, cfg.n_pad, cfg.k
    fmess = np.asarray(fmess, np.float32)
    bgraph = np.asarray(bgraph)

    shared = {
        "wrT": np.ascontiguousarray(W_r.T).astype(np.float16),
        "wz1T": np.ascontiguousarray(W_z[:, :h].T).astype(np.float16),
        "wh1T": np.ascontiguousarray(W_h[:, :h].T).astype(np.float16),
        "urT": np.ascontiguousarray(U_r.T).astype(np.float16),
        "wz2T": np.ascontiguousarray(W_z[:, h:].T).astype(np.float16),
        "wh2T": np.ascontiguousarray(W_h[:, h:].T).astype(np.float16),
        "ident16": np.eye(h, dtype=np.float16),
        "bz": np.asarray(b_z, np.float32).reshape(h, 1),
        "bur": np.asarray(b_Ur, np.float32).reshape(h, 1),
        "bh": np.asarray(b_h, np.float32).reshape(h, 1),
    }

    in_maps = []
    for c in range(cfg.n_cores):
        sl = slice(c * nl, (c + 1) * nl)
        fT = np.zeros((h, npad), np.float16)
        fT[:, :nl] = fmess[sl].T.astype(np.float16)
        bg = np.zeros((npad, k), np.int64)
        bg[:nl] = bgraph[sl]
        flat = bg.reshape(-1)                       # item stream, n-major
        # remap global rows into the split-AllGather table layout:
        # half A = rows [0,3136) of each core (8*3136 = 25088 rows),
        # half B = rows [3136,6250) of each core (8*3114 = 24912 rows)
        fc, fr = flat // nl, flat % nl
        flat = np.where(fr < 3136, fc * 3136 + fr,
                        25088 + fc * 3114 + (fr - 3136))
        pidx = (flat >> 1).astype(np.int16)
        idx = np.tile(pidx.reshape(cfg.items // 16, 16).T, (8, 1))
        mask = np.broadcast_to(
            (flat & 1).astype(np.uint8), (128, cfg.items)).copy()
        maskcol = np.ones((h, 1), np.float32)
        if c == 0:
            maskcol[:, 0] = 0.0
        in_maps.append({
            "fmessT": fT,
            "idx": idx,
            "mask": mask,
            "maskcol": maskcol,
            **shared,
        })
    return in_maps


def declare_io(nc, cfg: Cfg):
    h, npad = cfg.h, cfg.n_pad
    mk = lambda name, shape, dt: nc.dram_tensor(
        name, list(shape), dt, kind="ExternalInput").ap()
    ins = {
        "fmessT": mk("fmessT", (h, npad), F16),
        "idx": mk("idx", (128, cfg.items // 16), I16),
        "mask": mk("mask", (128, cfg.items), U8),
        "maskcol": mk("maskcol", (h, 1), F32),
        "wrT": mk("wrT", (h, h), F16),
        "wz1T": mk("wz1T", (h, h), F16),
        "wh1T": mk("wh1T", (h, h), F16),
        "urT": mk("urT", (h, h), F16),
        "wz2T": mk("wz2T", (h, h), F16),
        "wh2T": mk("wh2T", (h, h), F16),
        "ident16": mk("ident16", (h, h), F16),
        "bz": mk("bz", (h, 1), F32),
        "bur": mk("bur", (h, 1), F32),
        "bh": mk("bh", (h, 1), F32),
    }
    out = nc.dram_tensor("hT", [h, npad], F32, kind="ExternalOutput").ap()
    return ins, out


def build_gru(tc, out_hT, ins, cfg: Cfg):
    nc = tc.nc
    h, k, npad, nl = cfg.h, cfg.k, cfg.n_pad, cfg.n_loc
    N = cfg.n_mess
    rg = [list(range(cfg.n_cores))]

    tables = [nc.dram_tensor(f"table{i}", [N, h], F16, kind="Internal",
                             addr_space="Shared").ap() for i in range(2)]
    shard = nc.dram_tensor("shard", [nl, h], F16, kind="Internal").ap()
    tabs_pairs = [t.rearrange("(p two) hh -> p (two hh)", two=2)
                  for t in tables]

    with (
        tc.tile_pool(name="stat", bufs=1) as stat,
        tc.tile_pool(name="work", bufs=1) as work,
        tc.tile_pool(name="small", bufs=2) as small,
        tc.tile_pool(name="ps_r", bufs=2, space="PSUM") as ps_r,
        tc.tile_pool(name="ps_sm", bufs=1, space="PSUM") as ps_sm,
        tc.tile_pool(name="ps_tr", bufs=1, space="PSUM") as ps_tr,
    ):
        # ---- resident statics ----
        idx_sb = stat.tile([128, cfg.items // 16], I16)
        nc.sync.dma_start(idx_sb[:], ins["idx"][:])
        mask_sb = stat.tile([128, cfg.items], U8)
        nc.sync.dma_start(mask_sb[:], ins["mask"][:])
        rpre_sb = stat.tile([h, npad], F16)
        zpre_sb = stat.tile([h, npad], F16)
        hpre_sb = stat.tile([h, npad], F16)
        h16_full = stat.tile([h, npad], F16)
        w = {}
        for name in ("wrT", "wz1T", "wh1T", "wz2T", "wh2T", "urT",
                     "ident16"):
            w[name] = stat.tile([h, h], F16, tag=name, name=name)
            nc.sync.dma_start(w[name][:], ins[name][:])
        for name in ("bz", "bur", "bh", "maskcol"):
            w[name] = stat.tile([h, 1], F32, tag=name, name=name)
            nc.sync.dma_start(w[name][:], ins[name][:])
        urT, i16t = w["urT"], w["ident16"]

        def stt(out, in0, in1, op1):
            nc.vector.scalar_tensor_tensor(out, in0, 0.0, in1, ALU.bypass, op1)

        SPLIT = 3136            # half-A rows per core (14 chunks * 224)

        def shard_write_range(c0, c1):
            """Transpose h16_full cols [c0,c1) into shard rows [c0,c1)."""
            p = c0
            while p < c1:
                w_ = min(128, c1 - p)
                pst = ps_tr.tile([128, 128], F16)
                nc.tensor.transpose(pst[:w_, :], h16_full[:, p:p + w_],
                                    i16t[:])
                row = small.tile([128, 128], F16, tag="row")
                nc.scalar.activation(row[:w_, :], pst[:w_, :], AF.Copy)
                rows = max(0, min(nl - p, w_))
                if rows:
                    nc.sync.dma_start(shard[p:p + rows, :], row[:rows, :])
                p += w_

        # ---- phase 0: precomputes + step 1 (h == 0) ----
        for (n0, cn) in cfg.chunks:
            csl = slice(n0, n0 + cn)
            fsl = small.tile([h, cfg.cn], F16, tag="fsl")
            nc.sync.dma_start(fsl[:, :cn], ins["fmessT"][:, csl])
            fr = fsl[:, :cn]

            ps = ps_sm.tile([h, cfg.cn], F32, tag="psz")
            nc.tensor.matmul(ps[:, :cn], w["wrT"][:], fr,
                             start=True, stop=True)
            nc.scalar.activation(rpre_sb[:, csl], ps[:, :cn], AF.Copy)

            psz = ps_sm.tile([h, cfg.cn], F32, tag="psz")
            nc.tensor.matmul(psz[:, :cn], w["wz1T"][:], fr,
                             start=True, stop=True)
            nc.scalar.activation(zpre_sb[:, csl], psz[:, :cn], AF.Copy)
            z1 = small.tile([h, cfg.cn], F32, tag="z")
            nc.scalar.activation(z1[:, :cn], psz[:, :cn], AF.Sigmoid,
                                 bias=w["bz"][:])

            psh = ps_sm.tile([h, cfg.cn], F32, tag="psh")
            nc.tensor.matmul(psh[:, :cn], w["wh1T"][:], fr,
                             start=True, stop=True)
            nc.scalar.activation(hpre_sb[:, csl], psh[:, :cn], AF.Copy)
            ph1 = small.tile([h, cfg.cn], F32, tag="ph")
            nc.scalar.activation(ph1[:, :cn], psh[:, :cn], AF.Tanh,
                                 bias=w["bh"][:])

            hnew = small.tile([h, cfg.cn], F32, tag="hnew")
            stt(hnew[:, :cn], z1[:, :cn], ph1[:, :cn], ALU.mult)
            if n0 == 0:
                stt(hnew[:, 0:1], hnew[:, 0:1], w["maskcol"][:], ALU.mult)
            nc.scalar.activation(h16_full[:, csl], hnew[:, :cn], AF.Copy)

        shard_write_range(0, npad)
        nc.gpsimd.collective_compute(
            "AllGather", ALU.bypass, replica_groups=rg,
            ins=[shard[:SPLIT, :]], outs=[tables[0][:8 * SPLIT, :]])
        nc.gpsimd.collective_compute(
            "AllGather", ALU.bypass, replica_groups=rg,
            ins=[shard[SPLIT:nl, :]], outs=[tables[0][8 * SPLIT:N, :]])

        # ---- depth steps 2..depth ----
        gq = [0]
        for step in range(1, cfg.depth):
            last = step == cfg.depth - 1
            for ci, (n0, cn) in enumerate(cfg.chunks):
                csl = slice(n0, n0 + cn)
                citems = cn * k
                ioff = n0 * k
                cg = citems // 2

                pair = work.tile([128, 2, 2, cg], F16, tag="pair", bufs=3)
                for g in range(2):
                    nc.gpsimd.dma_gather(
                        out_ap=pair[:, g, :, :],
                        in_ap=tabs_pairs[(step - 1) % 2],
                        idxs_ap=idx_sb[:, (ioff + g * cg) // 16:
                                       (ioff + (g + 1) * cg) // 16],
                        num_idxs=cg,
                        num_idxs_reg=cg,
                        elem_size=2 * h,
                        transpose=True,
                        queue_num=gq[0] % 4,
                    )
                    gq[0] += 1
                lo = pair[:, :, 0, :]
                hi = pair[:, :, 1, :]
                msl = mask_sb[:, ioff:ioff + citems].rearrange(
                    "p (g c) -> p g c", g=2)

                # select: ACT copies lo, DVE overwrites hi where parity=1
                hn = work.tile([128, citems], F16, tag="hn", bufs=3)
                hn_g = hn[:, :citems].rearrange("p (g c) -> p g c", g=2)
                nc.scalar.activation(hn_g, lo, AF.Copy)
                nc.vector.copy_predicated(hn_g, msl, hi)

                # r2 = U_r @ hn (+ rpre broadcast); 4 bank-aligned
                # psum subtiles, stationary loads batched (urT x4 then
                # ident x4)
                sw = citems // 4         # 448
                nbw = sw // k            # 56
                psr_a = ps_r.tile([128, 2, 512], F32, tag="psr")
                psr_b = ps_r.tile([128, 2, 512], F32, tag="psr")
                sub = lambda j: (psr_a if j < 2 else psr_b)[:, j % 2, :sw]
                for j in range(4):
                    nc.tensor.matmul(sub(j), urT[:],
                                     hn[:, j * sw:(j + 1) * sw],
                                     start=True, stop=False)
                for j in range(4):
                    rb = rpre_sb[:, n0 + j * nbw:n0 + (j + 1) * nbw]
                    rb = rb.rearrange("p (a one) -> p a one", one=1)
                    rb = rb.broadcast_to((128, nbw, k))
                    nc.tensor.matmul(sub(j), i16t[:], rb,
                                     start=False, stop=True)
                r16 = work.tile([128, citems], F16, tag="r16", bufs=3)
                for j in range(4):
                    nc.scalar.activation(r16[:, j * sw:(j + 1) * sw],
                                         sub(j),
                                         AF.Sigmoid, bias=w["bur"][:])

                gated = work.tile([128, citems], F16, tag="gated", bufs=2)
                stt(gated[:, :citems], r16[:, :citems], hn[:, :citems],
                    ALU.mult)

                # sum_gated folded into W_h2: 8 accumulated matmuls
                gk = gated[:, :citems].rearrange("p (n kk) -> p n kk", kk=k)
                psh = ps_sm.tile([h, cfg.cn], F32, tag="psh")
                for kk in range(k):
                    nc.tensor.matmul(psh[:, :cn], w["wh2T"][:], gk[:, :, kk],
                                     start=(kk == 0), stop=False)
                nc.tensor.matmul(psh[:, :cn], i16t[:], hpre_sb[:, csl],
                                 start=False, stop=True)
                ph = small.tile([h, cfg.cn], F32, tag="ph")
                nc.scalar.activation(ph[:, :cn], psh[:, :cn], AF.Tanh,
                                     bias=w["bh"][:])

                # sum_h: stt halving tree over k == 8 (fp16 out)
                v = hn[:, :citems].rearrange("p (a two) -> p a two", two=2)
                t1 = work.tile([128, citems // 2], F16, tag="t1", bufs=2)
                stt(t1[:, :citems // 2], v[:, :, 0], v[:, :, 1], ALU.add)
                v = t1[:, :citems // 2].rearrange("p (a two) -> p a two",
                                                  two=2)
                t2 = work.tile([128, citems // 4], F16, tag="t2", bufs=2)
                stt(t2[:, :citems // 4], v[:, :, 0], v[:, :, 1], ALU.add)
                v = t2[:, :citems // 4].rearrange("p (a two) -> p a two",
                                                  two=2)
                sumh16 = small.tile([h, cfg.cn], F16, tag="sumh16")
                stt(sumh16[:, :cn], v[:, :, 0], v[:, :, 1], ALU.add)

                # z path
                psz = ps_sm.tile([h, cfg.cn], F32, tag="psz")
                nc.tensor.matmul(psz[:, :cn], w["wz2T"][:], sumh16[:, :cn],
                                 start=True, stop=False)
                nc.tensor.matmul(psz[:, :cn], i16t[:], zpre_sb[:, csl],
                                 start=False, stop=True)
                z = small.tile([h, cfg.cn], F32, tag="z")
                nc.scalar.activation(z[:, :cn], psz[:, :cn], AF.Sigmoid,
                                     bias=w["bz"][:])

                # h_new = sum_h + z * (pre_h - sum_h)
                t = small.tile([h, cfg.cn], F32, tag="tdiff")
                stt(t[:, :cn], ph[:, :cn], sumh16[:, :cn], ALU.subtract)
                tz = small.tile([h, cfg.cn], F32, tag="tz")
                stt(tz[:, :cn], t[:, :cn], z[:, :cn], ALU.mult)
                hnew = small.tile([h, cfg.cn], F32, tag="hnew")
                stt(hnew[:, :cn], tz[:, :cn], sumh16[:, :cn], ALU.add)
                if n0 == 0:
                    stt(hnew[:, 0:1], hnew[:, 0:1], w["maskcol"][:], ALU.mult)

                if last:
                    nc.sync.dma_start(out_hT[:, csl], hnew[:, :cn])
                else:
                    nc.scalar.activation(h16_full[:, csl], hnew[:, :cn],
                                         AF.Copy)
                    shard_write_range(n0, n0 + cn)
                    if n0 + cn == SPLIT:
                        nc.gpsimd.collective_compute(
                            "AllGather", ALU.bypass, replica_groups=rg,
                            ins=[shard[:SPLIT, :]],
                            outs=[tables[step % 2][:8 * SPLIT, :]])

            if not last:
                nc.gpsimd.collective_compute(
                    "AllGather", ALU.bypass, replica_groups=rg,
                    ins=[shard[SPLIT:nl, :]],
                    outs=[tables[step % 2][8 * SPLIT:N, :]])


CFG = Cfg()


_PROGRAM = None
LAST_RESULTS = None


def _get_program():
    global _PROGRAM
    if _PROGRAM is None:
        import concourse.bacc as bacc
        import concourse.tile as tile
        nc = bacc.Bacc("TRN2", target_bir_lowering=False, debug=False,
                       num_devices=CFG.n_cores, num_swdge_queues=4)
        ins, out = declare_io(nc, CFG)
        with tile.TileContext(nc) as tc:
            build_gru(tc, out, ins, CFG)
        nc.compile()
        _PROGRAM = nc
    return _PROGRAM


def kernel(fmess, bgraph, W_z, b_z, W_r, U_r, b_Ur, W_h, b_h, **_unused):
    global LAST_RESULTS
    import concourse.bass_utils as bass_utils
    cfg = CFG
    fmess_np = np.asarray(fmess)
    out_dtype = fmess_np.dtype
    in_maps = host_inputs(fmess_np, bgraph, W_z, b_z, W_r, U_r, b_Ur,
                          W_h, b_h, cfg)
    nc = _get_program()
    res = bass_utils.run_bass_kernel_spmd(
        nc, in_maps, core_ids=list(range(cfg.n_cores)))
    LAST_RESULTS = res
    parts = []
    for c in range(cfg.n_cores):
        hT = res.results[c]["hT"]
        parts.append(np.ascontiguousarray(hT[:, :cfg.n_loc].T))
    return np.concatenate(parts, axis=0).astype(out_dtype)


# revision 18
# speedup vs baseline: 1.4921x; 1.0473x over previous
"""DGCN-GRU message passing (nn_DGCNGRU) on 8 Trainium2 NeuronCores.

Strategy (sizes hardcoded for N=50000, K=8, H=128, DEPTH=5, 8 cores):
  - Messages sharded 6250 rows/core; small weights replicated; fp16 PE.
  - Evolving h lives in DRAM as a [50000, 128] fp16 row table, rebuilt by
    an fp16 AllGather of the 8 shards after every depth step.
  - Neighbor gather h[bgraph] via gpsimd dma_gather pair trick: idx =
    bgraph>>1 (int16 limit), each descriptor moves the 512B row pair,
    transpose=True lands the two candidate rows as two [128, idx] planes.
  - Pair select: ACT copies the lo plane, DVE copy_predicated overwrites
    with the hi plane under a uint8 parity mask (2 passes on 2 engines
    instead of 3 DVE stt passes).
  - All depth-invariant precomputes (W_r/W_z1/W_h1 @ fmess) are SBUF-
    resident fp16 for the whole kernel; no DRAM round trips per step.
  - Compute stays transposed [h on partitions, messages on free dim]:
    r2 = U_r @ hn in 4 PSUM subtiles then rpre broadcast-accumulated via
    identity matmuls (stationary reloads grouped: urT x4, ident x4,
    wh2 x8, wz2 x1, ident x2 per chunk); sigmoid/tanh on ACT with biases
    as per-partition operands; sum_h via a DVE stt halving tree (fp16);
    sum_gated folded into W_h2 (8 accumulated W_h2 matmuls).
  - Step 1 skips the gather (h == 0).

kernel(**inputs) takes full unsharded numpy inputs, returns the full
[50000, 128] float32 output. The Bass program is compiled once per
process and reused (it depends only on shapes).
"""


from dataclasses import dataclass

import numpy as np

import concourse.bass as bass
import concourse.mybir as mybir

F16 = mybir.dt.float16
F32 = mybir.dt.float32
U8 = mybir.dt.uint8
I16 = mybir.dt.int16
AF = mybir.ActivationFunctionType
ALU = mybir.AluOpType


@dataclass
class Cfg:
    n_mess: int = 50000
    n_cores: int = 8
    depth: int = 5
    k: int = 8
    h: int = 128
    cn: int = 224          # messages per chunk; 2 gathers of cn*k/2 idxs

    @property
    def n_loc(self):
        assert self.n_mess % self.n_cores == 0
        return self.n_mess // self.n_cores

    @property
    def n_pad(self):
        return ((self.n_loc + 127) // 128) * 128

    @property
    def items(self):
        return self.n_pad * self.k

    @property
    def chunks(self):
        out = []
        off = 0
        while off < self.n_pad:
            cn = min(self.cn, self.n_pad - off)
            assert (cn * self.k) % 128 == 0
            out.append((off, cn))
            off += cn
        return out


def host_inputs(fmess, bgraph, W_z, b_z, W_r, U_r, b_Ur, W_h, b_h, cfg: Cfg):
    n, h = cfg.n_mess, cfg.h
    nl, npad, k = cfg.n_loc, cfg.n_pad, cfg.k
    fmess = np.asarray(fmess, np.float32)
    bgraph = np.asarray(bgraph)

    shared = {
        "wrT": np.ascontiguousarray(W_r.T).astype(np.float16),
        "wz1T": np.ascontiguousarray(W_z[:, :h].T).astype(np.float16),
        "wh1T": np.ascontiguousarray(W_h[:, :h].T).astype(np.float16),
        "urT": np.ascontiguousarray(U_r.T).astype(np.float16),
        "wz2T": np.ascontiguousarray(W_z[:, h:].T).astype(np.float16),
        "wh2T": np.ascontiguousarray(W_h[:, h:].T).astype(np.float16),
        "ident16": np.eye(h, dtype=np.float16),
        "bz": np.asarray(b_z, np.float32).reshape(h, 1),
        "bur": np.asarray(b_Ur, np.float32).reshape(h, 1),
        "bh": np.asarray(b_h, np.float32).reshape(h, 1),
    }

    in_maps = []
    for c in range(cfg.n_cores):
        sl = slice(c * nl, (c + 1) * nl)
        fT = np.zeros((h, npad), np.float16)
        fT[:, :nl] = fmess[sl].T.astype(np.float16)
        bg = np.zeros((npad, k), np.int64)
        bg[:nl] = bgraph[sl]
        flat = bg.reshape(-1)                       # item stream, n-major
        # remap global rows into the split-AllGather table layout:
        # A = rows [0,3136) (8*3136=25088), B = [3136,5600) (8*2464=19712),
        # C = [5600,6250) (8*650=5200)
        fc, fr = flat // nl, flat % nl
        flat = np.where(
            fr < 3136, fc * 3136 + fr,
            np.where(fr < 5600, 25088 + fc * 2464 + (fr - 3136),
                     44800 + fc * 650 + (fr - 5600)))
        pidx = (flat >> 1).astype(np.int16)
        idx = np.tile(pidx.reshape(cfg.items // 16, 16).T, (8, 1))
        mask = np.broadcast_to(
            (flat & 1).astype(np.uint8), (128, cfg.items)).copy()
        maskcol = np.ones((h, 1), np.float32)
        if c == 0:
            maskcol[:, 0] = 0.0
        in_maps.append({
            "fmessT": fT,
            "idx": idx,
            "mask": mask,
            "maskcol": maskcol,
            **shared,
        })
    return in_maps


def declare_io(nc, cfg: Cfg):
    h, npad = cfg.h, cfg.n_pad
    mk = lambda name, shape, dt: nc.dram_tensor(
        name, list(shape), dt, kind="ExternalInput").ap()
    ins = {
        "fmessT": mk("fmessT", (h, npad), F16),
        "idx": mk("idx", (128, cfg.items // 16), I16),
        "mask": mk("mask", (128, cfg.items), U8),
        "maskcol": mk("maskcol", (h, 1), F32),
        "wrT": mk("wrT", (h, h), F16),
        "wz1T": mk("wz1T", (h, h), F16),
        "wh1T": mk("wh1T", (h, h), F16),
        "urT": mk("urT", (h, h), F16),
        "wz2T": mk("wz2T", (h, h), F16),
        "wh2T": mk("wh2T", (h, h), F16),
        "ident16": mk("ident16", (h, h), F16),
        "bz": mk("bz", (h, 1), F32),
        "bur": mk("bur", (h, 1), F32),
        "bh": mk("bh", (h, 1), F32),
    }
    out = nc.dram_tensor("hT", [h, npad], F32, kind="ExternalOutput").ap()
    return ins, out


def build_gru(tc, out_hT, ins, cfg: Cfg):
    nc = tc.nc
    h, k, npad, nl = cfg.h, cfg.k, cfg.n_pad, cfg.n_loc
    N = cfg.n_mess
    rg = [list(range(cfg.n_cores))]

    tables = [nc.dram_tensor(f"table{i}", [N, h], F16, kind="Internal",
                             addr_space="Shared").ap() for i in range(2)]
    shard = nc.dram_tensor("shard", [nl, h], F16, kind="Internal").ap()
    tabs_pairs = [t.rearrange("(p two) hh -> p (two hh)", two=2)
                  for t in tables]

    with (
        tc.tile_pool(name="stat", bufs=1) as stat,
        tc.tile_pool(name="work", bufs=1) as work,
        tc.tile_pool(name="small", bufs=2) as small,
        tc.tile_pool(name="ps_r", bufs=2, space="PSUM") as ps_r,
        tc.tile_pool(name="ps_sm", bufs=1, space="PSUM") as ps_sm,
        tc.tile_pool(name="ps_tr", bufs=1, space="PSUM") as ps_tr,
    ):
        # ---- resident statics ----
        idx_sb = stat.tile([128, cfg.items // 16], I16)
        nc.sync.dma_start(idx_sb[:], ins["idx"][:])
        mask_sb = stat.tile([128, cfg.items], U8)
        nc.sync.dma_start(mask_sb[:], ins["mask"][:])
        rpre_sb = stat.tile([h, npad], F16)
        zpre_sb = stat.tile([h, npad], F16)
        hpre_sb = stat.tile([h, npad], F16)
        h16_full = stat.tile([h, npad], F16)
        w = {}
        for name in ("wrT", "wz1T", "wh1T", "wz2T", "wh2T", "urT",
                     "ident16"):
            w[name] = stat.tile([h, h], F16, tag=name, name=name)
            nc.sync.dma_start(w[name][:], ins[name][:])
        for name in ("bz", "bur", "bh", "maskcol"):
            w[name] = stat.tile([h, 1], F32, tag=name, name=name)
            nc.sync.dma_start(w[name][:], ins[name][:])
        urT, i16t = w["urT"], w["ident16"]

        def stt(out, in0, in1, op1):
            nc.vector.scalar_tensor_tensor(out, in0, 0.0, in1, ALU.bypass, op1)

        # 3-way collective split: rows-per-core and table row offsets
        CUTS = [(0, 3136, 0), (3136, 5600, 25088), (5600, 6250, 44800)]

        def shard_write_range(c0, c1):
            """Transpose h16_full cols [c0,c1) into shard rows [c0,c1)."""
            p = c0
            while p < c1:
                w_ = min(128, c1 - p)
                pst = ps_tr.tile([128, 128], F16)
                nc.tensor.transpose(pst[:w_, :], h16_full[:, p:p + w_],
                                    i16t[:])
                row = small.tile([128, 128], F16, tag="row")
                nc.scalar.activation(row[:w_, :], pst[:w_, :], AF.Copy)
                rows = max(0, min(nl - p, w_))
                if rows:
                    nc.sync.dma_start(shard[p:p + rows, :], row[:rows, :])
                p += w_

        # ---- phase 0: precomputes + step 1 (h == 0) ----
        for (n0, cn) in cfg.chunks:
            csl = slice(n0, n0 + cn)
            fsl = small.tile([h, cfg.cn], F16, tag="fsl")
            nc.sync.dma_start(fsl[:, :cn], ins["fmessT"][:, csl])
            fr = fsl[:, :cn]

            ps = ps_sm.tile([h, cfg.cn], F32, tag="psz")
            nc.tensor.matmul(ps[:, :cn], w["wrT"][:], fr,
                             start=True, stop=True)
            nc.scalar.activation(rpre_sb[:, csl], ps[:, :cn], AF.Copy)

            psz = ps_sm.tile([h, cfg.cn], F32, tag="psz")
            nc.tensor.matmul(psz[:, :cn], w["wz1T"][:], fr,
                             start=True, stop=True)
            nc.scalar.activation(zpre_sb[:, csl], psz[:, :cn], AF.Copy)
            z1 = small.tile([h, cfg.cn], F32, tag="z")
            nc.scalar.activation(z1[:, :cn], psz[:, :cn], AF.Sigmoid,
                                 bias=w["bz"][:])

            psh = ps_sm.tile([h, cfg.cn], F32, tag="psh")
            nc.tensor.matmul(psh[:, :cn], w["wh1T"][:], fr,
                             start=True, stop=True)
            nc.scalar.activation(hpre_sb[:, csl], psh[:, :cn], AF.Copy)
            ph1 = small.tile([h, cfg.cn], F32, tag="ph")
            nc.scalar.activation(ph1[:, :cn], psh[:, :cn], AF.Tanh,
                                 bias=w["bh"][:])

            hnew = small.tile([h, cfg.cn], F32, tag="hnew")
            stt(hnew[:, :cn], z1[:, :cn], ph1[:, :cn], ALU.mult)
            if n0 == 0:
                stt(hnew[:, 0:1], hnew[:, 0:1], w["maskcol"][:], ALU.mult)
            nc.scalar.activation(h16_full[:, csl], hnew[:, :cn], AF.Copy)

        def do_cut(tbl, ci):
            r0, r1, t0 = CUTS[ci]
            sz = (r1 - r0) * cfg.n_cores
            nc.gpsimd.collective_compute(
                "AllGather", ALU.bypass, replica_groups=rg,
                ins=[shard[r0:r1, :]], outs=[tbl[t0:t0 + sz, :]])

        shard_write_range(0, npad)
        for ci in range(3):
            do_cut(tables[0], ci)

        # ---- depth steps 2..depth ----
        gq = [0]
        for step in range(1, cfg.depth):
            last = step == cfg.depth - 1
            for ci, (n0, cn) in enumerate(cfg.chunks):
                csl = slice(n0, n0 + cn)
                citems = cn * k
                ioff = n0 * k
                cg = citems // 2

                pair = work.tile([128, 2, 2, cg], F16, tag="pair", bufs=4)
                for g in range(2):
                    nc.gpsimd.dma_gather(
                        out_ap=pair[:, g, :, :],
                        in_ap=tabs_pairs[(step - 1) % 2],
                        idxs_ap=idx_sb[:, (ioff + g * cg) // 16:
                                       (ioff + (g + 1) * cg) // 16],
                        num_idxs=cg,
                        num_idxs_reg=cg,
                        elem_size=2 * h,
                        transpose=True,
                        queue_num=gq[0] % 4,
                    )
                    gq[0] += 1
                lo = pair[:, :, 0, :]
                hi = pair[:, :, 1, :]
                msl = mask_sb[:, ioff:ioff + citems].rearrange(
                    "p (g c) -> p g c", g=2)

                # select: ACT copies lo, DVE overwrites hi where parity=1
                hn = work.tile([128, citems], F16, tag="hn", bufs=3)
                hn_g = hn[:, :citems].rearrange("p (g c) -> p g c", g=2)
                nc.scalar.activation(hn_g, lo, AF.Copy)
                nc.vector.copy_predicated(hn_g, msl, hi)

                # r2 = U_r @ hn (+ rpre broadcast); 4 bank-aligned
                # psum subtiles, stationary loads batched (urT x4 then
                # ident x4)
                sw = citems // 4         # 448
                nbw = sw // k            # 56
                psr_a = ps_r.tile([128, 2, 512], F32, tag="psr")
                psr_b = ps_r.tile([128, 2, 512], F32, tag="psr")
                sub = lambda j: (psr_a if j < 2 else psr_b)[:, j % 2, :sw]
                for j in range(4):
                    nc.tensor.matmul(sub(j), urT[:],
                                     hn[:, j * sw:(j + 1) * sw],
                                     start=True, stop=False)
                for j in range(4):
                    rb = rpre_sb[:, n0 + j * nbw:n0 + (j + 1) * nbw]
                    rb = rb.rearrange("p (a one) -> p a one", one=1)
                    rb = rb.broadcast_to((128, nbw, k))
                    nc.tensor.matmul(sub(j), i16t[:], rb,
                                     start=False, stop=True)
                r16 = work.tile([128, citems], F16, tag="r16", bufs=3)
                for j in range(4):
                    nc.scalar.activation(r16[:, j * sw:(j + 1) * sw],
                                         sub(j),
                                         AF.Sigmoid, bias=w["bur"][:])

                gated = work.tile([128, citems], F16, tag="gated", bufs=2)
                stt(gated[:, :citems], r16[:, :citems], hn[:, :citems],
                    ALU.mult)

                # sum_gated folded into W_h2: 8 accumulated matmuls
                gk = gated[:, :citems].rearrange("p (n kk) -> p n kk", kk=k)
                psh = ps_sm.tile([h, cfg.cn], F32, tag="psh")
                for kk in range(k):
                    nc.tensor.matmul(psh[:, :cn], w["wh2T"][:], gk[:, :, kk],
                                     start=(kk == 0), stop=False)
                nc.tensor.matmul(psh[:, :cn], i16t[:], hpre_sb[:, csl],
                                 start=False, stop=True)
                ph = small.tile([h, cfg.cn], F32, tag="ph")
                nc.scalar.activation(ph[:, :cn], psh[:, :cn], AF.Tanh,
                                     bias=w["bh"][:])

                # sum_h: stt halving tree over k == 8 (fp16 out)
                v = hn[:, :citems].rearrange("p (a two) -> p a two", two=2)
                t1 = work.tile([128, citems // 2], F16, tag="t1", bufs=2)
                stt(t1[:, :citems // 2], v[:, :, 0], v[:, :, 1], ALU.add)
                v = t1[:, :citems // 2].rearrange("p (a two) -> p a two",
                                                  two=2)
                t2 = work.tile([128, citems // 4], F16, tag="t2", bufs=2)
                stt(t2[:, :citems // 4], v[:, :, 0], v[:, :, 1], ALU.add)
                v = t2[:, :citems // 4].rearrange("p (a two) -> p a two",
                                                  two=2)
                sumh16 = small.tile([h, cfg.cn], F16, tag="sumh16")
                stt(sumh16[:, :cn], v[:, :, 0], v[:, :, 1], ALU.add)

                # z path
                psz = ps_sm.tile([h, cfg.cn], F32, tag="psz")
                nc.tensor.matmul(psz[:, :cn], w["wz2T"][:], sumh16[:, :cn],
                                 start=True, stop=False)
                nc.tensor.matmul(psz[:, :cn], i16t[:], zpre_sb[:, csl],
                                 start=False, stop=True)
                z = small.tile([h, cfg.cn], F32, tag="z")
                nc.scalar.activation(z[:, :cn], psz[:, :cn], AF.Sigmoid,
                                     bias=w["bz"][:])

                # h_new = sum_h + z * (pre_h - sum_h)
                t = small.tile([h, cfg.cn], F32, tag="tdiff")
                stt(t[:, :cn], ph[:, :cn], sumh16[:, :cn], ALU.subtract)
                tz = small.tile([h, cfg.cn], F32, tag="tz")
                stt(tz[:, :cn], t[:, :cn], z[:, :cn], ALU.mult)
                if last:
                    hnew = small.tile([h, cfg.cn], F32, tag="hnew")
                    stt(hnew[:, :cn], tz[:, :cn], sumh16[:, :cn], ALU.add)
                    if n0 == 0:
                        stt(hnew[:, 0:1], hnew[:, 0:1], w["maskcol"][:],
                            ALU.mult)
                    nc.sync.dma_start(out_hT[:, csl], hnew[:, :cn])
                else:
                    stt(h16_full[:, csl], tz[:, :cn], sumh16[:, :cn],
                        ALU.add)
                    if n0 == 0:
                        stt(h16_full[:, 0:1], h16_full[:, 0:1],
                            w["maskcol"][:], ALU.mult)
                    shard_write_range(n0, n0 + cn)
                    if n0 + cn == 3136:
                        do_cut(tables[step % 2], 0)
                    elif n0 + cn == 5600:
                        do_cut(tables[step % 2], 1)

            if not last:
                do_cut(tables[step % 2], 2)


CFG = Cfg()


_PROGRAM = None
LAST_RESULTS = None


def _get_program():
    global _PROGRAM
    if _PROGRAM is None:
        import concourse.bacc as bacc
        import concourse.tile as tile
        nc = bacc.Bacc("TRN2", target_bir_lowering=False, debug=False,
                       num_devices=CFG.n_cores, num_swdge_queues=4)
        ins, out = declare_io(nc, CFG)
        with tile.TileContext(nc) as tc:
            build_gru(tc, out, ins, CFG)
        nc.compile()
        _PROGRAM = nc
    return _PROGRAM


def kernel(fmess, bgraph, W_z, b_z, W_r, U_r, b_Ur, W_h, b_h, **_unused):
    global LAST_RESULTS
    import concourse.bass_utils as bass_utils
    cfg = CFG
    fmess_np = np.asarray(fmess)
    out_dtype = fmess_np.dtype
    in_maps = host_inputs(fmess_np, bgraph, W_z, b_z, W_r, U_r, b_Ur,
                          W_h, b_h, cfg)
    nc = _get_program()
    res = bass_utils.run_bass_kernel_spmd(
        nc, in_maps, core_ids=list(range(cfg.n_cores)))
    LAST_RESULTS = res
    parts = []
    for c in range(cfg.n_cores):
        hT = res.results[c]["hT"]
        parts.append(np.ascontiguousarray(hT[:, :cfg.n_loc].T))
    return np.concatenate(parts, axis=0).astype(out_dtype)
